# revision 1
# baseline (speedup 1.0000x reference)
"""GAT (2-layer graph attention) Trainium2 kernel, 8-core SPMD.

Sharding: rows of N are sharded across 8 cores (512 rows each); every core
computes the full first-layer projections h (replicated) but only its own
rows' attention. Between layers the per-core x_cat rows are gathered on the
host and layer 2 runs as a second launch.

Math per layer (per batch b, head o):
    h = x @ W^T, f1 = x (W^T a1), f2 = x (W^T a2)
    P^T[j,i] = exp(lrelu(f1_i + f2_j + M[j,i]))   M = (adj^T-1)*3e4 (mask)
    [U | Z] = P^T.T @ [h | 1]  (PE accumulates over j-chunks)
    out_i = U_i / Z_i  (+ ELU for layer 1)

Layout: scores kept transposed [j(part), i(free)] so the PE contraction dim
(j) is on partitions for both operands; Z comes free via the ones column.
f2 is produced partition-major by tiny N=PAIR matmuls that reuse the already
loaded x^T weights; f1 (free-major) by M=1 matmuls + a ones outer-product.
"""

import os
from contextlib import ExitStack

import numpy as np
import ml_dtypes

import concourse.bacc as bacc
import concourse.tile as tile
import concourse.mybir as mybir
from concourse.bass_utils import run_bass_kernel_spmd

BF16 = mybir.dt.bfloat16
F32 = mybir.dt.float32
NPBF16 = ml_dtypes.bfloat16
AFT = mybir.ActivationFunctionType
ALU = mybir.AluOpType

NCORES = 8
B, N, F_IN, H, HID, OUT = 2, 4096, 512, 4, 128, 64
RPC = N // NCORES          # 512 rows per core
NJC = N // 128             # 32 column (j) chunks
NKC = F_IN // 128          # 4 contraction chunks for projections
NIS = RPC // 128           # 4 row (i) subtiles per core
MASKVAL = 30000.0
JGRP = 4                   # j-chunks per batched exp group
NGRP = NJC // JGRP
MSG = 8                    # ms per xT slab load

# engine-assignment config (tuned against TimelineSim)
CFG = {
    "tt_dve_mod": 5,        # every k-th u-build TT stays on DVE (rest gpsimd)
    "exp2_mod": 0,          # every k-th group: lrelu via max(exp(u),exp(.2u))
    "ts_act_mod": 0,        # every k-th chunk: f2-add on ACT Identity+bias
}

_cache = {}

last_exec_ns = []


def _build_layer(nheads, hid, label):
    nc = bacc.Bacc("TRN2", target_bir_lowering=False, debug=False,
                   enable_asserts=True, num_devices=NCORES)

    PAIR = 2 if nheads % 2 == 0 else 1
    NPR = nheads // PAIR   # head pairs
    HS = hid + 4           # hsb per-head block stride: [h | 1 | pad]
    RH = hid + 1           # attn rhs width: [h | 1]

    xT = nc.dram_tensor("xT", [B, F_IN, N], BF16, kind="ExternalInput")
    xT_own = nc.dram_tensor("xT_own", [B, F_IN, RPC], BF16, kind="ExternalInput")
    maskT = nc.dram_tensor("maskT", [B, N, RPC], BF16, kind="ExternalInput")
    NPR_ = nheads // (2 if nheads % 2 == 0 else 1)
    PAIR_ = nheads // NPR_
    w_aug = nc.dram_tensor("w_aug", [128, nheads * NKC * hid], BF16,
                           kind="ExternalInput")
    w1 = nc.dram_tensor("w1", [128, nheads * NKC], BF16, kind="ExternalInput")
    w2 = nc.dram_tensor("w2", [128, nheads * NKC], BF16, kind="ExternalInput")
    if label == "l1":
        out_d = nc.dram_tensor("xcat", [B, RPC, nheads * hid], BF16,
                               kind="ExternalOutput")
    else:
        out_d = nc.dram_tensor("out", [B, RPC, hid], F32, kind="ExternalOutput")

    with tile.TileContext(nc) as tc, ExitStack() as ctx:
        const_pool = ctx.enter_context(tc.tile_pool(name="const", bufs=1))
        mask_pool = ctx.enter_context(tc.tile_pool(name="mask", bufs=2))
        xt_pool = ctx.enter_context(tc.tile_pool(name="xt", bufs=2 * NKC))
        xto_pool = ctx.enter_context(tc.tile_pool(name="xto", bufs=2 * NKC))
        hsb_pool = ctx.enter_context(tc.tile_pool(name="hsb", bufs=NPR + 1))
        f2s_pool = ctx.enter_context(tc.tile_pool(name="f2s", bufs=NPR + 1))
        f1b_pool = ctx.enter_context(tc.tile_pool(name="f1b", bufs=2 * nheads + 1))
        f1r_pool = ctx.enter_context(tc.tile_pool(name="f1r", bufs=2))
        u_pool = ctx.enter_context(tc.tile_pool(name="u", bufs=4))
        w_pool = ctx.enter_context(tc.tile_pool(name="wb", bufs=4))
        p_pool = ctx.enter_context(tc.tile_pool(name="pb", bufs=3))
        eps_pool = ctx.enter_context(tc.tile_pool(name="eps", bufs=2))
        out_pool = ctx.enter_context(tc.tile_pool(name="outp", bufs=NIS))
        hpsum = ctx.enter_context(tc.tile_pool(name="hpsum", bufs=4, space="PSUM"))
        apsum = ctx.enter_context(tc.tile_pool(name="apsum", bufs=4, space="PSUM"))

        # constants: waug_sb holds [W^T_a | W^T_b] per (pair, kc); w2_sb holds
        # [w2_a | w2_b] per (pair, kc); w1_sb one col per (head, kc)
        waug_sb = const_pool.tile([128, NPR * NKC * PAIR * hid], BF16)
        w1_sb = const_pool.tile([128, nheads * NKC], BF16)
        w2_sb = const_pool.tile([128, NPR * NKC * PAIR], BF16)
        # host pre-bakes the SBUF layouts: one contiguous DMA each
        nc.sync.dma_start(waug_sb[:], w_aug.ap()[:, :])
        nc.sync.dma_start(w1_sb[:], w1.ap()[:, :])
        nc.sync.dma_start(w2_sb[:], w2.ap()[:, :])
        ones_sb = const_pool.tile([1, 128], BF16)
        nc.vector.memset(ones_sb[:], 1.0)

        # f1 for both batches upfront (keeps the b=1 boundary off the
        # serial f1 chain: M=1 matmuls -> copy -> outer-product -> copy)
        f1bs_all = []
        for b in range(B):
            xto = []
            for kc in range(NKC):
                t = xto_pool.tile([128, RPC], BF16, tag="xto")
                nc.sync.dma_start(t[:],
                                  xT_own.ap()[b, kc * 128:(kc + 1) * 128, :])
                xto.append(t)
            f1bs = []
            for o in range(nheads):
                fps = hpsum.tile([128, 512], F32, tag="hps", name="fps")
                for kc in range(NKC):
                    nc.tensor.matmul(fps[0:1, :RPC],
                                     w1_sb[:, o * NKC + kc:o * NKC + kc + 1],
                                     xto[kc][:, :],
                                     start=(kc == 0), stop=(kc == NKC - 1))
                f1r = f1r_pool.tile([1, RPC], BF16, tag="f1r")
                nc.vector.tensor_copy(f1r[:], fps[0:1, :RPC])
                bps = hpsum.tile([128, 512], F32, tag="hps")
                nc.tensor.matmul(bps[:, :RPC], ones_sb[:, :], f1r[:, :],
                                 start=True, stop=True)
                f1b = f1b_pool.tile([128, RPC], BF16, tag="f1b")
                nc.scalar.copy(f1b[:], bps[:, :RPC])
                f1bs.append(f1b)
            f1bs_all.append(f1bs)

        for b in range(B):
            f1bs = f1bs_all[b]
            masksb = mask_pool.tile([128, NJC * RPC], BF16, tag="masksb")
            NQ = NJC // 4

            def load_mask_quarter(mq):
                nc.sync.dma_start(
                    masksb[:, mq * NQ * RPC:(mq + 1) * NQ * RPC].rearrange(
                        "p (c i) -> p c i", i=RPC),
                    maskT.ap()[b, mq * NQ * 128:(mq + 1) * NQ * 128].rearrange(
                        "(c p) i -> p c i", p=128))

            load_mask_quarter(0)

            # --- projections: h (pairs) + f2 (partition-major) ---
            hsbs = []    # per pair: [128, NJC*PAIR*HS]
            for pr in range(NPR):
                hs = hsb_pool.tile([128, NJC * PAIR * HS], BF16, tag="hs")
                hv = hs[:].rearrange("p (c q f) -> p c q f", q=PAIR, f=HS)
                nc.vector.memset(hv[:, :, :, hid:hid + 1], 1.0)
                hsbs.append(hs)
            f2ps = [hpsum.tile([128, 512], F32, tag="hps", name="f2ps")
                    for _ in range(NPR)]
            f2sbs = [f2s_pool.tile([128, NJC * PAIR], F32, tag="f2sb",
                                   name="f2sb")
                     for _ in range(NPR)]
            for ms in range(NJC):
                if ms == MSG:
                    for _mq in range(1, 4):
                        load_mask_quarter(_mq)
                if ms % MSG == 0:
                    xslabs = []
                    for kc in range(NKC):
                        xs = xt_pool.tile([128, MSG * 128], BF16, tag="xt")
                        nc.sync.dma_start(
                            xs[:],
                            xT.ap()[b, kc * 128:(kc + 1) * 128,
                                    ms * 128:(ms + MSG) * 128])
                        xslabs.append(xs)
                hps = [hpsum.tile([128, 512], F32, tag="hps", name="hps")
                       for _ in range(NPR)]
                for kc in range(NKC):
                    xt_t = xslabs[kc][:, (ms % MSG) * 128:(ms % MSG + 1) * 128]
                    st = (kc == 0)
                    sp = (kc == NKC - 1)
                    for pr in range(NPR):
                        i = (pr * NKC + kc) * PAIR
                        nc.tensor.matmul(
                            hps[pr][:, :PAIR * hid], xt_t,
                            waug_sb[:, i * hid:(i + PAIR) * hid],
                            start=st, stop=sp)
                        nc.tensor.matmul(
                            f2ps[pr][:, ms * PAIR:(ms + 1) * PAIR], xt_t,
                            w2_sb[:, i:i + PAIR],
                            start=st, stop=sp)
                for pr in range(NPR):
                    hv = hsbs[pr][:].rearrange("p (c q f) -> p c q f",
                                               q=PAIR, f=HS)
                    pv = hps[pr][:, :PAIR * hid].rearrange(
                        "p (q f) -> p q f", f=hid)
                    nc.scalar.copy(hv[:, ms, :, 0:hid], pv)
                # f2 to fp32 SBUF incrementally so attention can pipeline
                if ms % JGRP == JGRP - 1:
                    lo = (ms - (JGRP - 1)) * PAIR
                    hi = (ms + 1) * PAIR
                    for pr in range(NPR):
                        nc.vector.tensor_copy(f2sbs[pr][:, lo:hi],
                                              f2ps[pr][:, lo:hi])

            # --- attention per head ---
            if label == "l1":
                xcts = [out_pool.tile([128, nheads * hid], BF16, tag="xct",
                                      name="xct")
                        for _ in range(NIS)]
            for o in range(nheads):
                pr, q = o // PAIR, o % PAIR
                hs = hsbs[pr]
                f2sb = f2sbs[pr]

                def blk(jc):
                    return (jc * PAIR + q) * HS

                aps = [apsum.tile([128, 512], F32, tag="aps", name="aps")
                       for _ in range(NIS)]
                for g in range(NGRP):
                    wb = w_pool.tile([128, JGRP * RPC], BF16, tag="wb")
                    ub = u_pool.tile([128, JGRP * RPC], BF16, tag="ub")
                    for j8 in range(JGRP):
                        jc = g * JGRP + j8
                        usl = ub[:, j8 * RPC:(j8 + 1) * RPC]
                        gidx = ((b * nheads + o) * NGRP + g) * JGRP + j8
                        m = CFG["tt_dve_mod"]
                        tte = nc.vector if (m and gidx % m == 0) else nc.gpsimd
                        tte.tensor_add(usl, masksb[:, jc * RPC:(jc + 1) * RPC],
                                       f1bs[o][:])
                        f2ap = f2sb[:, jc * PAIR + q:jc * PAIR + q + 1]
                        ma = CFG["ts_act_mod"]
                        if ma and gidx % ma == 0:
                            nc.scalar.activation(usl, usl, AFT.Identity,
                                                 bias=f2ap)
                        else:
                            nc.vector.tensor_scalar(usl, usl, f2ap, None,
                                                    ALU.add)
                    gctr = (b * nheads + o) * NGRP + g
                    e2 = CFG["exp2_mod"]
                    pb = p_pool.tile([128, JGRP * RPC], BF16, tag="pb")
                    if e2 and gctr % e2 == 0:
                        # exp(lrelu(u)) = max(exp(u), exp(0.2u)): trades the
                        # 1x DVE lrelu-STT for a second batched ACT exp plus
                        # a 2x-mode TT max (wb slot holds exp(u))
                        nc.scalar.activation(wb[:], ub[:], AFT.Exp)
                        nc.scalar.activation(pb[:], ub[:], AFT.Exp, scale=0.2)
                        nc.vector.tensor_tensor(pb[:], wb[:], pb[:], ALU.max)
                    else:
                        # lrelu: one batched STT on DVE per group
                        nc.vector.scalar_tensor_tensor(
                            wb[:], ub[:], 0.2, ub[:], ALU.mult, ALU.max)
                        nc.scalar.activation(pb[:], wb[:], AFT.Exp)
                    for j8 in range(JGRP):
                        jc = g * JGRP + j8
                        rhs = hs[:, blk(jc):blk(jc) + RH]
                        for s in range(NIS):
                            nc.tensor.matmul(
                                aps[s][:, :RH],
                                pb[:, j8 * RPC + s * 128:
                                   j8 * RPC + (s + 1) * 128],
                                rhs,
                                start=(jc == 0), stop=(jc == NJC - 1))
                # epilogue
                for s in range(NIS):
                    zr = eps_pool.tile([128, 1], F32, tag="zr")
                    nc.vector.reciprocal(zr[:], aps[s][:, hid:hid + 1])
                    if label == "l1":
                        tt = eps_pool.tile([128, hid], BF16, tag="tt")
                        nc.scalar.activation(tt[:], aps[s][:, :hid], AFT.Copy,
                                             bias=0.0, scale=zr[:])
                        tm = eps_pool.tile([128, hid], BF16, tag="tm")
                        nc.vector.tensor_scalar(tm[:], tt[:], 0.0, None,
                                                ALU.min)
                        te = eps_pool.tile([128, hid], BF16, tag="te")
                        nc.scalar.activation(te[:], tm[:], AFT.Exp)
                        tr = eps_pool.tile([128, hid], BF16, tag="tr")
                        nc.vector.tensor_scalar(tr[:], tt[:], 0.0, -1.0,
                                                ALU.max, ALU.add)
                        nc.vector.tensor_add(
                            xcts[s][:, o * hid:(o + 1) * hid], te[:], tr[:])
                        if o == nheads - 1:
                            nc.sync.dma_start(
                                out_d.ap()[b, s * 128:(s + 1) * 128, :],
                                xcts[s][:])
                    else:
                        ot = out_pool.tile([128, hid], F32, tag="ot")
                        nc.scalar.activation(ot[:], aps[s][:, :hid], AFT.Copy,
                                             bias=0.0, scale=zr[:])
                        nc.sync.dma_start(
                            out_d.ap()[b, s * 128:(s + 1) * 128, :], ot[:])

    nc.compile()
    return nc


def _get_programs():
    if "l1" not in _cache:
        _cache["l1"] = _build_layer(H, HID, "l1")
    if "l2" not in _cache:
        _cache["l2"] = _build_layer(1, OUT, "l2")
    return _cache["l1"], _cache["l2"]


def _aug_weights(W, a1, a2, nheads, hid):
    """Bake W^T / w1 / w2 into the exact SBUF layouts the kernel loads."""
    W = W.reshape(nheads, hid, F_IN).astype(np.float32)
    a1 = a1.reshape(nheads, hid).astype(np.float32)
    a2 = a2.reshape(nheads, hid).astype(np.float32)
    w1 = np.einsum("ohf,oh->of", W, a1)   # [o, F_IN]
    w2 = np.einsum("ohf,oh->of", W, a2)
    PAIR = 2 if nheads % 2 == 0 else 1
    NPR = nheads // PAIR
    waug = np.zeros((128, nheads * NKC * hid), np.float32)
    w1c = np.zeros((128, nheads * NKC), np.float32)
    w2c = np.zeros((128, nheads * NKC), np.float32)
    for o in range(nheads):
        pr, q = o // PAIR, o % PAIR
        WT = W[o].T  # [F_IN, hid]
        for kc in range(NKC):
            sl = slice(kc * 128, (kc + 1) * 128)
            i = (pr * NKC + kc) * PAIR + q
            waug[:, i * hid:(i + 1) * hid] = WT[sl]
            w1c[:, o * NKC + kc] = w1[o, sl]
            w2c[:, i] = w2[o, sl]
    return (waug.astype(NPBF16), w1c.astype(NPBF16), w2c.astype(NPBF16))


def _run(nc, in_maps):
    trace = bool(int(os.environ.get("GAT_TRACE", "0")))
    res = run_bass_kernel_spmd(nc, in_maps, list(range(NCORES)), trace=trace)
    if res.exec_time_ns is not None:
        last_exec_ns.append(res.exec_time_ns)
    return res


def kernel(**inputs):
    global last_exec_ns
    last_exec_ns = []
    x = np.asarray(inputs["x"], np.float32)
    adj = np.asarray(inputs["adj"])
    W_heads = np.asarray(inputs["W_heads"], np.float32)
    a1_heads = np.asarray(inputs["a1_heads"], np.float32)
    a2_heads = np.asarray(inputs["a2_heads"], np.float32)
    W_out = np.asarray(inputs["W_out"], np.float32)
    a1_out = np.asarray(inputs["a1_out"], np.float32)
    a2_out = np.asarray(inputs["a2_out"], np.float32)

    nc1, nc2 = _get_programs()

    xT = np.ascontiguousarray(x.transpose(0, 2, 1)).astype(NPBF16)  # [B,F,N]
    waug1, w11, w21 = _aug_weights(W_heads, a1_heads, a2_heads, H, HID)
    waug2, w12, w22 = _aug_weights(W_out[None], a1_out[None], a2_out[None],
                                   1, OUT)

    masks = []
    for c in range(NCORES):
        sl = slice(c * RPC, (c + 1) * RPC)
        m = (adj[:, sl, :].transpose(0, 2, 1).astype(np.float32) - 1.0) * MASKVAL
        masks.append(np.ascontiguousarray(m).astype(NPBF16))

    in_maps1 = []
    for c in range(NCORES):
        sl = slice(c * RPC, (c + 1) * RPC)
        in_maps1.append({
            "xT": xT,
            "xT_own": np.ascontiguousarray(xT[:, :, sl]),
            "maskT": masks[c],
            "w_aug": waug1,
            "w1": w11,
            "w2": w21,
        })
    r1 = _run(nc1, in_maps1)
    xcat = np.concatenate(
        [r1.results[c]["xcat"].astype(np.float32) for c in range(NCORES)],
        axis=1)  # [B, N, H*HID]
    xcatT = np.ascontiguousarray(xcat.transpose(0, 2, 1)).astype(NPBF16)

    in_maps2 = []
    for c in range(NCORES):
        sl = slice(c * RPC, (c + 1) * RPC)
        in_maps2.append({
            "xT": xcatT,
            "xT_own": np.ascontiguousarray(xcatT[:, :, sl]),
            "maskT": masks[c],
            "w_aug": waug2,
            "w1": w12,
            "w2": w22,
        })
    r2 = _run(nc2, in_maps2)
    out = np.concatenate(
        [r2.results[c]["out"] for c in range(NCORES)], axis=1)
    return out.astype(np.float32)



# revision 49
# speedup vs baseline: 1.1142x; 1.1142x over previous
"""GAT (2-layer graph attention) Trainium2 kernel, 8-core SPMD.

Sharding: batch-split — cores 0-3 take batch 0, cores 4-7 batch 1; within a
group each core owns 1024 rows (i) and attends against all N=4096 columns (j).

Score factorization (the key trick): with s = f1_i + f2_j,
    exp(lrelu(s)) = max(exp(s), exp(0.2 s))
                  = max(E1_j*E1'_i, E2_j*E2'_i)
where E1 = exp(f2), E1' = exp(f1), E2/E2' the 0.2-scaled variants.  The O(N^2)
exp of the naive formulation becomes O(N) exps plus rank-1 products, and the
adjacency enters as a plain 0/1 multiply:
    p[j,i] = adjT[j,i] * max(m1, m2).

Per (head, j-chunk) unit the 4 elementwise passes are distributed across
engines by a static flavor map (tuned against TimelineSim):
  F2p: ACT builds m1/m2 (Copy w/ per-partition scale), Pool max, DVE mult
  F2h: ACT m1 + DVE TS m2, Pool max, DVE mult
  F6:  DVE TS builds (4x mode), Pool max, DVE mult
  F2/F1: ACT-or-DVE builds with DVE max, DVE mult
[U | Z] then accumulates on PE via the ones-column trick as usual.
"""

import os
from contextlib import ExitStack

import numpy as np
import ml_dtypes

import concourse.bacc as bacc
import concourse.tile as tile
import concourse.mybir as mybir
from concourse.bass_utils import run_bass_kernel_spmd

BF16 = mybir.dt.bfloat16
F32 = mybir.dt.float32
NPBF16 = ml_dtypes.bfloat16
AFT = mybir.ActivationFunctionType
ALU = mybir.AluOpType

NCORES = 8
B, N, F_IN, H, HID, OUT = 2, 4096, 512, 4, 128, 64
CPG = NCORES // B          # cores per batch group
RPC = N // CPG             # 1024 own rows per core
NIS = RPC // 128           # 8 row (i) subtiles per core
NJC = N // 128             # 32 column (j) chunks
NKC = F_IN // 128          # 4 contraction chunks for projections
MSG = 4                    # ms chunks per xT slab load
PREB = 11                  # max score tiles built ahead of consumption
LAG = 3                    # software-pipeline lag between build stages

# flavor cycle: F5 = PE outers + Pool max, F2 = ACT builds + DVE max,
# F6 = DVE TS builds + Pool max.  All end with a DVE mask-mult.
FLAVORS = ["F2p", "F1", "F2p", "F1", "F2p", "F6", "F1", "F2p"]

_cache = {}

last_exec_ns = []


def _build_layer(nheads, hid, label):
    nc = bacc.Bacc("TRN2", target_bir_lowering=False, debug=False,
                   enable_asserts=True, num_devices=NCORES)

    PAIR = 2 if nheads % 2 == 0 else 1
    NPR = nheads // PAIR   # head pairs
    HS = hid + 2           # per (jc, head) block stride in hsb: [h | 1 | pad]
    RH = hid + 1           # attn rhs width: [h | 1]
    PH = PAIR * hid        # proj cols per pair

    PROJ = (label != "l2a")
    adjT = nc.dram_tensor("adjT", [N, RPC], BF16, kind="ExternalInput")
    if PROJ:
        xT = nc.dram_tensor("xT", [F_IN, N], BF16, kind="ExternalInput")
        xT_own = nc.dram_tensor("xT_own", [F_IN, RPC], BF16,
                                kind="ExternalInput")
        w_aug = nc.dram_tensor("w_aug", [128, nheads * NKC * hid], BF16,
                               kind="ExternalInput")
        w1 = nc.dram_tensor("w1", [128, nheads * NKC], BF16,
                            kind="ExternalInput")
        w2 = nc.dram_tensor("w2", [128, nheads * NKC], BF16,
                            kind="ExternalInput")
    else:
        hsb_in = nc.dram_tensor("hsb_in", [128, NJC * HS], BF16,
                                kind="ExternalInput")
        f2_in = nc.dram_tensor("f2_in", [128, NJC], F32,
                               kind="ExternalInput")
        f1_in = nc.dram_tensor("f1_in", [1, RPC], BF16,
                               kind="ExternalInput")
    if label == "l1":
        out_d = nc.dram_tensor("xcat", [RPC, nheads * hid], BF16,
                               kind="ExternalOutput")
    else:
        out_d = nc.dram_tensor("out", [RPC, hid], F32, kind="ExternalOutput")

    # static flavor map over (head, jc): early units (prebuilt during the
    # ACT-heavy projection phase) avoid ACT; later units lean on it.
    EARLY = ["F1", "F6", "F1", "F6", "F1", "F6", "F1", "F1"]
    LATE = ["F2p", "F1", "F2p", "F1", "F2p", "F6", "F2p", "F1"]
    fl = {}
    k = 0
    for o in range(nheads):
        for jc in range(NJC):
            if label != "l1":
                fl[(o, jc)] = FLAVORS[k % len(FLAVORS)]
            elif k < 3 * PREB:
                fl[(o, jc)] = EARLY[k % len(EARLY)]
            else:
                fl[(o, jc)] = LATE[k % len(LATE)]
            k += 1

    with tile.TileContext(nc) as tc, ExitStack() as ctx:
        const_pool = ctx.enter_context(tc.tile_pool(name="const", bufs=1))
        adj_pool = ctx.enter_context(tc.tile_pool(name="adj", bufs=1))
        xt_pool = ctx.enter_context(tc.tile_pool(name="xt", bufs=2 * NKC))
        xto_pool = ctx.enter_context(tc.tile_pool(name="xto", bufs=NKC))
        hsb_pool = ctx.enter_context(tc.tile_pool(name="hsb", bufs=1))
        eb_pool = ctx.enter_context(tc.tile_pool(name="eb", bufs=2 * nheads))
        ec_pool = ctx.enter_context(tc.tile_pool(name="ec", bufs=3 * NPR))
        f1r_pool = ctx.enter_context(tc.tile_pool(name="f1r", bufs=2))
        mb_pool = ctx.enter_context(tc.tile_pool(name="mb", bufs=6))
        q_pool = ctx.enter_context(tc.tile_pool(name="qb", bufs=5))
        p_pool = ctx.enter_context(tc.tile_pool(name="pb", bufs=PREB + 4))
        eps_pool = ctx.enter_context(tc.tile_pool(name="eps", bufs=8))
        out_pool = ctx.enter_context(tc.tile_pool(name="outp", bufs=NIS))
        # PSUM: scratch 4 + f2 1 + aps 3 = 8 banks exactly
        scr = ctx.enter_context(tc.tile_pool(
            name="scr", bufs=2 if label == "l1" else 4, space="PSUM"))
        f2psum = ctx.enter_context(tc.tile_pool(name="f2ps", bufs=1,
                                                space="PSUM"))
        apsum = ctx.enter_context(tc.tile_pool(
            name="aps", bufs=5 if label == "l1" else 3, space="PSUM"))

        # ---- constants ----
        if PROJ:
            waug_sb = const_pool.tile([128, NPR * NKC * PAIR * hid], BF16)
            w1_sb = const_pool.tile([128, nheads * NKC], BF16)
            w2_sb = const_pool.tile([128, NPR * NKC * PAIR], BF16)
            nc.sync.dma_start(waug_sb[:], w_aug.ap()[:, :])
            nc.sync.dma_start(w1_sb[:], w1.ap()[:, :])
            nc.sync.dma_start(w2_sb[:], w2.ap()[:, :])
        ones_sb = const_pool.tile([1, 128], BF16)
        nc.vector.memset(ones_sb[:], 1.0)

        # ---- adjacency (0/1 bf16), quarters staggered into the proj loop ----
        adjsb = adj_pool.tile([128, NJC * RPC], BF16)
        NQ = NJC // 4

        def load_adj_quarter(mq):
            nc.sync.dma_start(
                adjsb[:, mq * NQ * RPC:(mq + 1) * NQ * RPC].rearrange(
                    "p (c i) -> p c i", i=RPC),
                adjT.ap()[mq * NQ * 128:(mq + 1) * NQ * 128].rearrange(
                    "(c p) i -> p c i", p=128))

        # ---- own-column x slabs (for f1) ----
        if PROJ:
            xto = []
            for kc in range(NKC):
                t = xto_pool.tile([128, RPC], BF16, tag="xto")
                nc.sync.dma_start(t[:],
                                  xT_own.ap()[kc * 128:(kc + 1) * 128, :])
                xto.append(t)

        # ---- f1 per head -> E1pb/E2pb broadcast exp tiles [128, RPC] ----
        e1pb, e2pb = [], []
        for o in range(nheads):
            f1r = f1r_pool.tile([1, RPC], BF16, tag="f1r")
            if PROJ:
                for hf in range(RPC // 512):
                    fps = scr.tile([128, 512], F32, tag="scr", name="fps")
                    for kc in range(NKC):
                        nc.tensor.matmul(
                            fps[0:1, :],
                            w1_sb[:, o * NKC + kc:o * NKC + kc + 1],
                            xto[kc][:, hf * 512:(hf + 1) * 512],
                            start=(kc == 0), stop=(kc == NKC - 1))
                    if hf == 0:
                        nc.vector.tensor_copy(
                            f1r[:, hf * 512:(hf + 1) * 512], fps[0:1, :])
                    else:
                        nc.scalar.copy(f1r[:, hf * 512:(hf + 1) * 512],
                                       fps[0:1, :])
            else:
                nc.sync.dma_start(f1r[:], f1_in.ap()[:, :])
            e1 = eb_pool.tile([128, RPC], BF16, tag="eb", name="e1")
            e2 = eb_pool.tile([128, RPC], BF16, tag="eb", name="e2")
            for hf in range(RPC // 512):
                bps = scr.tile([128, 512], F32, tag="scr", name="bps")
                nc.tensor.matmul(bps[:, :], ones_sb[:, :],
                                 f1r[:, hf * 512:(hf + 1) * 512],
                                 start=True, stop=True)
                sl = slice(hf * 512, (hf + 1) * 512)
                nc.scalar.activation(e1[:, sl], bps[:, :], AFT.Exp)
                nc.scalar.activation(e2[:, sl], bps[:, :], AFT.Exp, scale=0.2)
            e1pb.append(e1)
            e2pb.append(e2)

        # ---- projection: h (pairs) + f2 ----
        hsb = hsb_pool.tile([128, NJC * nheads * HS], BF16)
        hv = hsb[:].rearrange("p (c o f) -> p c o f", o=nheads, f=HS)
        if PROJ:
            nc.vector.memset(hv[:, :, :, hid:hid + 1], 1.0)
            f2ps = f2psum.tile([128, 512], F32)  # pr-blocks col pr*NJC*PAIR
            nc.vector.memset(f2ps[:], 0.0)
        else:
            nc.sync.dma_start(hsb[:], hsb_in.ap()[:, :])

        def f2col(pr):
            return pr * NJC * PAIR

        # E-col tiles (filled incrementally per 8-ms group)
        w = NJC * PAIR
        e1c = [ec_pool.tile([128, w], F32, tag="ec", name="e1c")
               for _ in range(NPR)]
        e2c = [ec_pool.tile([128, w], F32, tag="ec", name="e2c")
               for _ in range(NPR)]
        if not PROJ:
            f2sb_in = ec_pool.tile([128, NJC], F32, tag="ecf", name="f2sb_in")
            nc.sync.dma_start(f2sb_in[:], f2_in.ap()[:, :])
            nc.scalar.activation(e1c[0][:], f2sb_in[:], AFT.Exp)
            nc.scalar.activation(e2c[0][:], f2sb_in[:], AFT.Exp, scale=0.2)

        # ---- attention unit machinery (emitted interleaved with proj) ----
        # aps slot layout: 3 psum tiles, slots (3, 3, 2), RH-wide each
        SLOT = [(0, 0), (0, 1), (0, 2), (1, 0), (1, 1), (1, 2), (2, 0), (2, 1)]

        if label == "l1":
            xcts = [out_pool.tile([128, nheads * hid], BF16, tag="xct",
                                  name="xct")
                    for _ in range(NIS)]

        units = [(o, jc) for o in range(nheads) for jc in range(NJC)]
        ustate = {}            # (o,jc) -> dict of built tiles per stage
        aps_of = {}            # o -> aps tile list

        def stage_a(u):
            """builds m1/m2 on ACT and/or DVE, per flavor."""
            o, jc = u
            pr, qh = o // PAIR, o % PAIR
            jq = jc * PAIR + qh
            f = fl[u]
            m1 = mb_pool.tile([128, RPC], BF16, tag="mb", name="m1")
            m2 = mb_pool.tile([128, RPC], BF16, tag="mb", name="m2")
            if f in ("F2", "F2p"):            # both builds on ACT
                nc.scalar.activation(m1[:], e1pb[o][:], AFT.Copy, bias=0.0,
                                     scale=e1c[pr][:, jq:jq + 1])
                nc.scalar.activation(m2[:], e2pb[o][:], AFT.Copy, bias=0.0,
                                     scale=e2c[pr][:, jq:jq + 1])
            elif f == "F2h":                  # split ACT / DVE
                nc.scalar.activation(m1[:], e1pb[o][:], AFT.Copy, bias=0.0,
                                     scale=e1c[pr][:, jq:jq + 1])
                nc.vector.tensor_scalar(m2[:], e2pb[o][:],
                                        e2c[pr][:, jq:jq + 1], None, ALU.mult)
            else:                             # F6 / F1: both on DVE
                nc.vector.tensor_scalar(m1[:], e1pb[o][:],
                                        e1c[pr][:, jq:jq + 1], None, ALU.mult)
                nc.vector.tensor_scalar(m2[:], e2pb[o][:],
                                        e2c[pr][:, jq:jq + 1], None, ALU.mult)
            ustate[u] = {"f": f, "m1": m1, "m2": m2}

        def stage_b(u):
            """max: always DVE (Pool/ACT cannot run TT-max)."""
            st = ustate[u]
            qt = q_pool.tile([128, RPC], BF16, tag="qb", name="qt")
            nc.vector.tensor_tensor(qt[:], st["m1"][:], st["m2"][:], ALU.max)
            st["qt"] = qt
            del st["m1"], st["m2"]

        def stage_c(u):
            """mask mult: Pool for F2p/F2h/F6 flavors, DVE otherwise."""
            o, jc = u
            st = ustate[u]
            p = p_pool.tile([128, RPC], BF16, tag="pb", name="p")
            eng = nc.gpsimd if st["f"] in ("F2p", "F2h", "F6") else nc.vector
            eng.tensor_tensor(p[:], st["qt"][:],
                              adjsb[:, jc * RPC:(jc + 1) * RPC],
                              ALU.mult)
            st["p"] = p
            del st["qt"]

        def stage_mm(u):
            """aps matmuls"""
            o, jc = u
            pr, qh = o // PAIR, o % PAIR
            if o not in aps_of:
                aps_of[o] = [apsum.tile([128, 512], F32, tag="aps",
                                        name="aps") for _ in range(3)]
                for t_ in aps_of[o]:
                    nc.vector.memset(t_[:], 0.0)
            aps = aps_of[o]
            hb = (jc * nheads + pr * PAIR + qh) * HS
            rhs = hsb[:, hb:hb + RH]
            p = ustate[u]["p"]
            for s in range(NIS):
                t, i = SLOT[s]
                nc.tensor.matmul(
                    aps[t][:, i * RH:i * RH + RH],
                    p[:, s * 128:(s + 1) * 128], rhs,
                    start=False, stop=(jc == NJC - 1),
                    skip_group_check=True)
            del ustate[u]

        def epilogue(o):
            aps = aps_of[o]
            for s in range(NIS):
                t, i = SLOT[s]
                ap = aps[t][:, i * RH:i * RH + RH]
                zr = eps_pool.tile([128, 1], F32, tag="zr")
                nc.vector.reciprocal(zr[:], ap[:, hid:hid + 1])
                if label == "l1":
                    tt = eps_pool.tile([128, hid], BF16, tag="tt")
                    nc.scalar.activation(tt[:], ap[:, :hid], AFT.Copy,
                                         bias=0.0, scale=zr[:])
                    tm = eps_pool.tile([128, hid], BF16, tag="tm")
                    nc.vector.tensor_scalar(tm[:], tt[:], 0.0, None, ALU.min)
                    te = eps_pool.tile([128, hid], BF16, tag="te")
                    nc.scalar.activation(te[:], tm[:], AFT.Exp)
                    tr = eps_pool.tile([128, hid], BF16, tag="tr")
                    nc.vector.tensor_scalar(tr[:], tt[:], 0.0, -1.0,
                                            ALU.max, ALU.add)
                    nc.vector.tensor_add(
                        xcts[s][:, o * hid:(o + 1) * hid], te[:], tr[:])
                    if o == nheads - 1:
                        nc.sync.dma_start(
                            out_d.ap()[s * 128:(s + 1) * 128, :], xcts[s][:])
                else:
                    ot = out_pool.tile([128, hid], F32, tag="ot")
                    nc.scalar.activation(ot[:], ap[:, :hid], AFT.Copy,
                                         bias=0.0, scale=zr[:])
                    nc.sync.dma_start(
                        out_d.ap()[s * 128:(s + 1) * 128, :], ot[:])
            del aps_of[o]

        # 4-stage pipeline cursors over the unit list
        ia = ib = ic = im = 0

        def pump_build(max_jc, n):
            """advance build stages (a/b/c) by up to n steps each, keeping
            1-unit lags between stages; stage_a only enters units whose
            E-cols (jc <= max_jc) are already emitted; p tiles in flight
            bounded by PREB."""
            nonlocal ia, ib, ic
            for _ in range(n):
                moved = False
                if (ia < len(units) and units[ia][1] <= max_jc
                        and ia - ib <= LAG and ic - im < PREB):
                    stage_a(units[ia]); ia += 1; moved = True
                if ib < ia and ib - ic <= LAG:
                    stage_b(units[ib]); ib += 1; moved = True
                if ic < ib and ic - im < PREB:
                    stage_c(units[ic]); ic += 1; moved = True
                if not moved:
                    break

        def pump_mm():
            """consume all fully-built units in order."""
            nonlocal im
            while im < ic:
                u = units[im]
                stage_mm(u)
                im += 1
                if u[1] == NJC - 1:
                    epilogue(u[0])

        # ---- projection loop with interleaved E-exps and score prebuild ----
        load_adj_quarter(0)
        EGRP = 8               # ms group size for incremental E-col exps
        for ms in (range(NJC) if PROJ else []):
            if ms % 8 == 6 and ms // 8 < 3:
                load_adj_quarter(ms // 8 + 1)
            if ms % MSG == 0:
                xslabs = []
                for kc in range(NKC):
                    xs = xt_pool.tile([128, MSG * 128], BF16, tag="xt")
                    nc.sync.dma_start(
                        xs[:],
                        xT.ap()[kc * 128:(kc + 1) * 128,
                                ms * 128:(ms + MSG) * 128])
                    xslabs.append(xs)
            hps = [scr.tile([128, 512], F32, tag="scr", name="hp")
                   for _ in range(NPR)]
            for kc in range(NKC):
                xt_t = xslabs[kc][:, (ms % MSG) * 128:(ms % MSG + 1) * 128]
                st = (kc == 0)
                sp = (kc == NKC - 1)
                for pr in range(NPR):
                    i = (pr * NKC + kc) * PAIR
                    nc.tensor.matmul(
                        hps[pr][:, 0:PH], xt_t,
                        waug_sb[:, i * hid:(i + PAIR) * hid],
                        start=st, stop=sp)
                    nc.tensor.matmul(
                        f2ps[:, f2col(pr) + ms * PAIR:
                             f2col(pr) + (ms + 1) * PAIR], xt_t,
                        w2_sb[:, i:i + PAIR],
                        start=False, stop=sp, skip_group_check=True)
            for pr in range(NPR):
                hsrc = hps[pr][:, 0:PH].rearrange("p (o f) -> p o f", f=hid)
                nc.scalar.activation(
                    hv[:, ms, pr * PAIR:(pr + 1) * PAIR, 0:hid], hsrc,
                    AFT.Copy, bias=0.0, scale=1.0)
            if ms % EGRP == EGRP - 1:
                g0 = (ms - (EGRP - 1)) * PAIR
                g1 = (ms + 1) * PAIR
                for pr in range(NPR):
                    sl = slice(f2col(pr) + g0, f2col(pr) + g1)
                    nc.scalar.activation(e1c[pr][:, g0:g1], f2ps[:, sl],
                                         AFT.Exp)
                    nc.scalar.activation(e2c[pr][:, g0:g1], f2ps[:, sl],
                                         AFT.Exp, scale=0.2)
                # prebuild scores for ready chunks, bounded by p capacity
                pump_build(ms, PREB)

        if not PROJ:
            for mq in range(1, 4):
                load_adj_quarter(mq)

        # ---- main attention drain ----
        while im < len(units):
            pump_build(NJC - 1, 1)
            pump_mm()

    nc.compile()
    return nc


def _build_l2p():
    """Tiny projection launch for layer 2: each core projects only its own
    1024 rows of xcat -> h2/f2/f1; host gathers into the l2a inputs."""
    nc = bacc.Bacc("TRN2", target_bir_lowering=False, debug=False,
                   enable_asserts=True, num_devices=NCORES)
    HS = OUT + 2
    NC8 = RPC // 128  # 8 own chunks
    xT_own = nc.dram_tensor("xT_own", [F_IN, RPC], BF16, kind="ExternalInput")
    w_aug = nc.dram_tensor("w_aug", [128, NKC * OUT], BF16,
                           kind="ExternalInput")
    w1 = nc.dram_tensor("w1", [128, NKC], BF16, kind="ExternalInput")
    w2 = nc.dram_tensor("w2", [128, NKC], BF16, kind="ExternalInput")
    hsb_o = nc.dram_tensor("hsb_o", [128, NC8 * HS], BF16,
                           kind="ExternalOutput")
    f2_o = nc.dram_tensor("f2_o", [128, NC8], F32, kind="ExternalOutput")
    f1_o = nc.dram_tensor("f1_o", [1, RPC], BF16, kind="ExternalOutput")

    with tile.TileContext(nc) as tc, ExitStack() as ctx:
        pool = ctx.enter_context(tc.tile_pool(name="sb", bufs=NKC))
        ps = ctx.enter_context(tc.tile_pool(name="ps", bufs=2, space="PSUM"))
        waug_sb = pool.tile([128, NKC * OUT], BF16)
        w1_sb = pool.tile([128, NKC], BF16)
        w2_sb = pool.tile([128, NKC], BF16)
        nc.sync.dma_start(waug_sb[:], w_aug.ap()[:, :])
        nc.sync.dma_start(w1_sb[:], w1.ap()[:, :])
        nc.sync.dma_start(w2_sb[:], w2.ap()[:, :])
        xto = []
        for kc in range(NKC):
            t = pool.tile([128, RPC], BF16, tag="xto")
            nc.sync.dma_start(t[:], xT_own.ap()[kc * 128:(kc + 1) * 128, :])
            xto.append(t)

        # f1 (own rows)
        f1sb = pool.tile([1, RPC], BF16)
        for hf in range(RPC // 512):
            fps = ps.tile([128, 512], F32, tag="f1p", name="fps")
            for kc in range(NKC):
                nc.tensor.matmul(fps[0:1, :], w1_sb[:, kc:kc + 1],
                                 xto[kc][:, hf * 512:(hf + 1) * 512],
                                 start=(kc == 0), stop=(kc == NKC - 1))
            nc.vector.tensor_copy(f1sb[:, hf * 512:(hf + 1) * 512],
                                  fps[0:1, :])
        nc.sync.dma_start(f1_o.ap()[:, :], f1sb[:])

        # h2 + f2 for the 8 own chunks
        hp = ps.tile([128, 512], F32, tag="hp")     # 8 x 64 cols
        fp = ps.tile([128, 512], F32, tag="fp")     # 8 f2 cols
        nc.vector.memset(hp[:], 0.0)
        nc.vector.memset(fp[:, 0:NC8], 0.0)
        for c in range(NC8):
            for kc in range(NKC):
                xt_t = xto[kc][:, c * 128:(c + 1) * 128]
                nc.tensor.matmul(hp[:, c * OUT:(c + 1) * OUT], xt_t,
                                 waug_sb[:, kc * OUT:(kc + 1) * OUT],
                                 start=False, stop=(kc == NKC - 1),
                                 skip_group_check=True)
                nc.tensor.matmul(fp[:, c:c + 1], xt_t, w2_sb[:, kc:kc + 1],
                                 start=False, stop=(kc == NKC - 1),
                                 skip_group_check=True)
        hsb = pool.tile([128, NC8 * HS], BF16)
        hv = hsb[:].rearrange("p (c f) -> p c f", f=HS)
        nc.vector.memset(hv[:, :, OUT:HS], 1.0)
        for c in range(NC8):
            nc.vector.tensor_copy(hv[:, c, 0:OUT],
                                  hp[:, c * OUT:(c + 1) * OUT])
        f2sb = pool.tile([128, NC8], F32)
        nc.vector.tensor_copy(f2sb[:], fp[:, 0:NC8])
        nc.sync.dma_start(hsb_o.ap()[:, :], hsb[:])
        nc.sync.dma_start(f2_o.ap()[:, :], f2sb[:])

    nc.compile()
    return nc


def _get_programs():
    if "l1" not in _cache:
        _cache["l1"] = _build_layer(H, HID, "l1")
    if "l2p" not in _cache:
        _cache["l2p"] = _build_l2p()
    if "l2" not in _cache:
        _cache["l2"] = _build_layer(1, OUT, "l2a")
    return _cache["l1"], _cache["l2"]


def _aug_weights(W, a1, a2, nheads, hid):
    """Bake W^T / w1 / w2 into the exact SBUF layouts the kernel loads."""
    W = W.reshape(nheads, hid, F_IN).astype(np.float32)
    a1 = a1.reshape(nheads, hid).astype(np.float32)
    a2 = a2.reshape(nheads, hid).astype(np.float32)
    w1 = np.einsum("ohf,oh->of", W, a1)   # [o, F_IN]
    w2 = np.einsum("ohf,oh->of", W, a2)
    PAIR = 2 if nheads % 2 == 0 else 1
    waug = np.zeros((128, nheads * NKC * hid), np.float32)
    w1c = np.zeros((128, nheads * NKC), np.float32)
    w2c = np.zeros((128, nheads * NKC), np.float32)
    for o in range(nheads):
        pr, q = o // PAIR, o % PAIR
        WT = W[o].T  # [F_IN, hid]
        for kc in range(NKC):
            sl = slice(kc * 128, (kc + 1) * 128)
            i = (pr * NKC + kc) * PAIR + q
            waug[:, i * hid:(i + 1) * hid] = WT[sl]
            w1c[:, o * NKC + kc] = w1[o, sl]
            w2c[:, i] = w2[o, sl]
    return (waug.astype(NPBF16), w1c.astype(NPBF16), w2c.astype(NPBF16))


def _run(nc, in_maps):
    trace = bool(int(os.environ.get("GAT_TRACE", "0")))
    res = run_bass_kernel_spmd(nc, in_maps, list(range(NCORES)), trace=trace)
    if res.exec_time_ns is not None:
        last_exec_ns.append(res.exec_time_ns)
    return res


def kernel(**inputs):
    global last_exec_ns
    last_exec_ns = []
    x = np.asarray(inputs["x"], np.float32)
    adj = np.asarray(inputs["adj"])
    W_heads = np.asarray(inputs["W_heads"], np.float32)
    a1_heads = np.asarray(inputs["a1_heads"], np.float32)
    a2_heads = np.asarray(inputs["a2_heads"], np.float32)
    W_out = np.asarray(inputs["W_out"], np.float32)
    a1_out = np.asarray(inputs["a1_out"], np.float32)
    a2_out = np.asarray(inputs["a2_out"], np.float32)

    nc1, nc2 = _get_programs()

    xT = np.ascontiguousarray(x.transpose(0, 2, 1)).astype(NPBF16)  # [B,F,N]
    waug1, w11, w21 = _aug_weights(W_heads, a1_heads, a2_heads, H, HID)
    waug2, w12, w22 = _aug_weights(W_out[None], a1_out[None], a2_out[None],
                                   1, OUT)
    id128 = np.eye(128, dtype=NPBF16)

    adjs = []
    for c in range(NCORES):
        b, sl = c // CPG, slice((c % CPG) * RPC, (c % CPG + 1) * RPC)
        a = adj[b, sl, :].T.astype(np.float32)  # [N, RPC] 0/1
        adjs.append(np.ascontiguousarray(a).astype(NPBF16))

    def maps_for(xTb_list):
        m = []
        for c in range(NCORES):
            b, sl = c // CPG, slice((c % CPG) * RPC, (c % CPG + 1) * RPC)
            xTb = xTb_list[b]
            m.append({
                "xT": xTb,
                "xT_own": np.ascontiguousarray(xTb[:, sl]),
                "adjT": adjs[c],
                "id128": id128,
            })
        return m

    in_maps1 = maps_for([xT[0], xT[1]])
    for mm in in_maps1:
        mm.update({"w_aug": waug1, "w1": w11, "w2": w21})
    r1 = _run(nc1, in_maps1)
    xcat = np.stack(
        [np.concatenate([r1.results[b * CPG + g]["xcat"].astype(np.float32)
                         for g in range(CPG)], axis=0) for b in range(B)])
    xcatT = np.ascontiguousarray(xcat.transpose(0, 2, 1)).astype(NPBF16)

    # l2p: project own rows only, then gather h2/f2 per batch group
    in_mapsP = []
    for c in range(NCORES):
        b, sl = c // CPG, slice((c % CPG) * RPC, (c % CPG + 1) * RPC)
        in_mapsP.append({
            "xT_own": np.ascontiguousarray(xcatT[b][:, sl]),
            "w_aug": waug2, "w1": w12, "w2": w22,
        })
    rp = _run(_cache["l2p"], in_mapsP)
    hsb_full, f2_full = [], []
    for b in range(B):
        hsb_full.append(np.concatenate(
            [rp.results[b * CPG + g]["hsb_o"] for g in range(CPG)], axis=1))
        f2_full.append(np.concatenate(
            [rp.results[b * CPG + g]["f2_o"] for g in range(CPG)], axis=1))
    in_maps2 = []
    for c in range(NCORES):
        b = c // CPG
        in_maps2.append({
            "adjT": adjs[c],
            "hsb_in": np.ascontiguousarray(hsb_full[b]),
            "f2_in": np.ascontiguousarray(f2_full[b]),
            "f1_in": rp.results[c]["f1_o"],
        })
    r2 = _run(nc2, in_maps2)
    out = np.stack(
        [np.concatenate([r2.results[b * CPG + g]["out"] for g in range(CPG)],
                        axis=0) for b in range(B)])
    return out.astype(np.float32)
